# revision 15
# baseline (speedup 1.0000x reference)
"""Trainium2 Bass kernel for a single transformer encoder layer.

Problem: B=4, S=2048, D=512, H=8 (dk=64), DFF=2048, f32 I/O.
Sharding: 8 cores = (batch b, token-half). Each core computes the full
layer for its 1024 tokens; K/V are computed for the whole 2048-token
batch on both cores of a pair (duplicated, zero communication).

Layout strategy (per core):
  - activations enter feature-major (xT, host-pretransposed, bf16)
  - K^T, Q^T feature-major with head-pairs stacked on 128 partitions
  - V token-major, stored per head with an appended ones column [V_h|1]
  - scores computed transposed sT[t2, t1] (2-head row-packed matmuls
    into one 2-bank PSUM tile, one wide EXP per t2-tile)
  - ctx^T via V-stationary matmuls; the ones column makes row 64 of the
    ctx accumulator the softmax denominator Z (no separate Z matmuls)
  - 1/Z batched per t1-block: ln+exp(-x) on ACT (same table set as the
    softmax EXP), partition-broadcast via a tiny K=8 matmul
  - odd heads' normalized ctx^T moved to partitions 64-127 by SBUF DMA
  - Wo -> token-major attn_out, residual+LN1 (bn_stats; rstd=exp(-.5 ln v))
  - PE-transpose x1 -> FFN1 (bias+relu fused on DVE) -> FFN2
  - residual+LN2 -> out (token-major f32)
  - ScalarE runs ONLY Exp/Ln (one activation-table set, no reloads);
    all other evictions/elementwise run on DVE or GpSimd.
"""

from contextlib import ExitStack

import numpy as np
import ml_dtypes

import concourse.bass as bass
import concourse.tile as tile
from concourse import mybir, bacc
from concourse.bass_utils import run_bass_kernel_spmd
from concourse.masks import make_identity

F32 = mybir.dt.float32
BF16 = mybir.dt.bfloat16
AF = mybir.ActivationFunctionType
OP = mybir.AluOpType

B, S, D = 4, 2048, 512
H, DK, DFF = 8, 64, 2048
EPS = 1e-5
P = 128
T1 = 1024          # own tokens per core
NCORES = 8

KD = D // P        # 4   d-tiles
NT2 = S // P       # 16  t2 tiles (context tokens)
NT1 = T1 // P      # 8   t1 tiles (own tokens)
NPAIR = H // 2     # 4   head pairs
NDFF = DFF // P    # 16  dff tiles
NB1 = T1 // 512    # 2   own-token 512-blocks
NBS = S // 512     # 4   context 512-blocks
DV1 = DK + 1       # 65  V columns incl the ones column


def emit(ctx: ExitStack, tc, io):
    nc = tc.nc

    xT, xTo, xo = io["xT"], io["xTo"], io["xo"]
    wq, wk, wv, wo, w1, w2 = io["wq"], io["wk"], io["wv"], io["wo"], io["w1"], io["w2"]
    out = io["out"]

    const = ctx.enter_context(tc.tile_pool(name="const", bufs=1))
    persist = ctx.enter_context(tc.tile_pool(name="persist", bufs=1))
    exp_pool = ctx.enter_context(tc.tile_pool(name="exp", bufs=4))
    cxu_pool = ctx.enter_context(tc.tile_pool(name="cxu", bufs=10))
    work = ctx.enter_context(tc.tile_pool(name="work", bufs=2))
    stat = ctx.enter_context(tc.tile_pool(name="stat", bufs=4))
    norm = ctx.enter_context(tc.tile_pool(name="norm", bufs=1))
    xo_pool = ctx.enter_context(tc.tile_pool(name="xo", bufs=2))
    out_pool = ctx.enter_context(tc.tile_pool(name="out", bufs=2))

    mm_ps = ctx.enter_context(tc.tile_pool(name="mm_ps", bufs=2, space="PSUM"))
    sc_ps = ctx.enter_context(tc.tile_pool(name="sc_ps", bufs=2, space="PSUM"))
    ctx_ps = ctx.enter_context(tc.tile_pool(name="ctx_ps", bufs=1, space="PSUM"))

    # ---- constants ----
    ident_sb = const.tile([P, P], BF16)
    make_identity(nc, ident_sb[:])
    eps_sb = const.tile([P, 1], F32)
    nc.vector.memset(eps_sb[:], EPS)
    # indicator for the 1/Z partition-broadcast: row h covers columns of head h
    ind8_sb = const.tile([8, D], BF16)
    nc.sync.dma_start(ind8_sb[:], io["ind8"][:, :])

    # per-partition bias tiles (feature-major evictions)
    bqt = const.tile([P, KD], F32)
    nc.sync.dma_start(bqt[:], io["bq"][:].rearrange("(m p) -> p m", p=P))
    bkt = const.tile([P, KD], F32)
    nc.sync.dma_start(bkt[:], io["bk"][:].rearrange("(m p) -> p m", p=P))
    b1t = const.tile([P, NDFF], F32)
    nc.sync.dma_start(b1t[:], io["b1"][:].rearrange("(m p) -> p m", p=P))

    # free-axis broadcast tiles (token-major ops)
    def bc_tile(name):
        t = const.tile([P, D], BF16, tag=f"bc_{name}")
        a = io[name][:]
        bcast = bass.AP(tensor=a.tensor, offset=a.offset, ap=[[0, P]] + list(a.ap))
        nc.gpsimd.dma_start(t[:], bcast)
        return t

    bvb = bc_tile("bv")
    bob = bc_tile("bo")
    b2b = bc_tile("b2")
    g1b = bc_tile("g1")
    be1b = bc_tile("be1")
    g2b = bc_tile("g2")
    be2b = bc_tile("be2")

    # ---- persistent SBUF arrays ----
    xT_sb = persist.tile([P, KD, S], BF16, tag="xT")
    for k in range(KD):
        nc.sync.dma_start(
            xT_sb[:, k, :], xT[:, :].rearrange("(k p) t -> p k t", p=P)[:, k, :]
        )
    xTo_sb = persist.tile([P, KD, T1], BF16, tag="xTo")
    for k in range(KD):
        nc.sync.dma_start(
            xTo_sb[:, k, :], xTo[:, :].rearrange("(k p) t -> p k t", p=P)[:, k, :]
        )

    wq_sb = persist.tile([P, KD, D], BF16, tag="wq")
    nc.sync.dma_start(wq_sb[:], wq[:, :].rearrange("(k p) m -> p k m", p=P))
    wk_sb = persist.tile([P, KD, D], BF16, tag="wk")
    nc.sync.dma_start(wk_sb[:], wk[:, :].rearrange("(k p) m -> p k m", p=P))
    wv_sb = persist.tile([P, KD, D], BF16, tag="wv")
    nc.sync.dma_start(wv_sb[:], wv[:, :].rearrange("(k p) m -> p k m", p=P))
    wo_sb = persist.tile([P, KD, D], BF16, tag="wo")
    nc.sync.dma_start(wo_sb[:], wo[:, :].rearrange("(k p) m -> p k m", p=P))

    kt_sb = persist.tile([P, NPAIR, S], BF16, tag="kt")
    qt_sb = persist.tile([P, NPAIR, T1], BF16, tag="qt")
    # V with per-head ones column: [t2 128, t2tile, head, 65]
    ve_sb = persist.tile([P, NT2, H, DV1], BF16, tag="ve")
    nc.vector.memset(ve_sb[:, :, :, DK:DV1], 1.0)
    ctxT_sb = persist.tile([P, NPAIR, T1], BF16, tag="ctxT")
    x1_sb = persist.tile([P, NT1, D], BF16, tag="x1")
    x1T_sb = persist.tile([P, KD, T1], BF16, tag="x1T")
    h1T_sb = persist.tile([P, NDFF, T1], BF16, tag="h1T")

    # ---- projections ----
    # K^T (feature-major, head-pairs stacked): [dk-pair 128, t2]
    for m in range(KD):
        for nb in range(NBS):
            ps = mm_ps.tile([P, 512], F32, tag="mm")
            for k in range(KD):
                nc.tensor.matmul(
                    ps[:],
                    wk_sb[:, k, m * P:(m + 1) * P],
                    xT_sb[:, k, nb * 512:(nb + 1) * 512],
                    start=(k == 0),
                    stop=(k == KD - 1),
                )
            nc.vector.tensor_scalar_add(
                kt_sb[:, m, nb * 512:(nb + 1) * 512], ps[:], bkt[:, m:m + 1]
            )
    # Q^T
    for m in range(KD):
        for nb in range(NB1):
            ps = mm_ps.tile([P, 512], F32, tag="mm")
            for k in range(KD):
                nc.tensor.matmul(
                    ps[:],
                    wq_sb[:, k, m * P:(m + 1) * P],
                    xTo_sb[:, k, nb * 512:(nb + 1) * 512],
                    start=(k == 0),
                    stop=(k == KD - 1),
                )
            nc.vector.tensor_scalar_add(
                qt_sb[:, m, nb * 512:(nb + 1) * 512], ps[:], bqt[:, m:m + 1]
            )
    # V (token-major, per-head strided into [h, 65] groups): [t2 128, dv]
    for i in range(NT2):
        ps = mm_ps.tile([P, 512], F32, tag="mm")
        for k in range(KD):
            nc.tensor.matmul(
                ps[:],
                xT_sb[:, k, i * P:(i + 1) * P],
                wv_sb[:, k, :],
                start=(k == 0),
                stop=(k == KD - 1),
            )
        nc.vector.tensor_tensor(
            ve_sb[:, i, :, 0:DK],
            ps[:].rearrange("p (h d) -> p h d", h=H),
            bvb[:].rearrange("p (h d) -> p h d", h=H),
            OP.add,
        )

    # W1 shares the xT slot (xT dead after projections); prefetch during attention
    w1_sb = persist.tile([P, KD, DFF], BF16, tag="xT")
    nc.sync.dma_start(w1_sb[:], w1[:, :].rearrange("(k p) m -> p k m", p=P))
    # W2 shares the xTo slot (dead after Q projection)
    w2_sb = persist.tile([P, NDFF, D], BF16, tag="xTo")
    nc.sync.dma_start(w2_sb[:], w2[:, :].rearrange("(k p) m -> p k m", p=P))

    def layer_norm(r, gb, beb, dest, eng=None):
        """dest = LN(r)*g + be; r is f32 SBUF [128, D]."""
        eng = eng or nc.gpsimd
        st = stat.tile([P, 6], F32, tag="st")
        nc.vector.bn_stats(st[:], r[:])
        mv = stat.tile([P, 2], F32, tag="mv")
        nc.vector.bn_aggr(mv[:], st[:])
        lnv = stat.tile([P, 1], F32, tag="lnv")
        nc.scalar.activation(lnv[:], mv[:, 1:2], AF.Ln, bias=eps_sb[:, 0:1])
        rstd = stat.tile([P, 1], F32, tag="rstd")
        nc.scalar.activation(rstd[:], lnv[:], AF.Exp, scale=-0.5)
        xc = work.tile([P, D], F32, tag="xc")
        nc.vector.tensor_scalar(
            xc[:], r[:], mv[:, 0:1], rstd[:], op0=OP.subtract, op1=OP.mult
        )
        xg = work.tile([P, D], F32, tag="xg")
        eng.tensor_tensor(xg[:], xc[:], gb[:], OP.mult)
        eng.tensor_tensor(dest, xg[:], beb[:], OP.add)

    def post_attn(t1t):
        """Wo + residual + LN1 + transpose(x1) for one 128-token tile."""
        ao = mm_ps.tile([P, 512], F32, tag="mm")
        for k in range(NPAIR):
            nc.tensor.matmul(
                ao[:],
                ctxT_sb[:, k, t1t * P:(t1t + 1) * P],
                wo_sb[:, k, :],
                start=(k == 0),
                stop=(k == NPAIR - 1),
            )
        xo_t = xo_pool.tile([P, D], F32)
        nc.sync.dma_start(xo_t[:], xo[t1t * P:(t1t + 1) * P, :])
        r = work.tile([P, D], F32, tag="r")
        nc.vector.tensor_tensor(r[:], ao[:], xo_t[:], OP.add)
        nc.gpsimd.tensor_tensor(r[:], r[:], bob[:], OP.add)
        layer_norm(r, g1b, be1b, x1_sb[:, t1t, :])
        for j in range(KD):
            tp = mm_ps.tile([P, P], BF16, tag="mm")
            nc.tensor.transpose(
                tp[:], x1_sb[:, t1t, j * P:(j + 1) * P], ident_sb[:]
            )
            nc.vector.tensor_copy(x1T_sb[:, j, t1t * P:(t1t + 1) * P], tp[:])

    def ffn1(t1b):
        for m in range(NDFF):
            ps = mm_ps.tile([P, 512], F32, tag="mm")
            for k in range(KD):
                nc.tensor.matmul(
                    ps[:],
                    w1_sb[:, k, m * P:(m + 1) * P],
                    x1T_sb[:, k, t1b * 512:(t1b + 1) * 512],
                    start=(k == 0),
                    stop=(k == KD - 1),
                )
            # h1 = relu(ps + b1) fused on DVE
            nc.vector.tensor_scalar(
                h1T_sb[:, m, t1b * 512:(t1b + 1) * 512], ps[:],
                b1t[:, m:m + 1], 0.0, op0=OP.add, op1=OP.max,
            )

    def ffn2(t1t):
        ff = mm_ps.tile([P, 512], F32, tag="mm")
        for k in range(NDFF):
            nc.tensor.matmul(
                ff[:],
                h1T_sb[:, k, t1t * P:(t1t + 1) * P],
                w2_sb[:, k, :],
                start=(k == 0),
                stop=(k == NDFF - 1),
            )
        r = work.tile([P, D], F32, tag="r2")
        nc.vector.tensor_tensor(r[:], ff[:], x1_sb[:, t1t, :], OP.add)
        nc.vector.tensor_tensor(r[:], r[:], b2b[:], OP.add)
        o = out_pool.tile([P, D], F32)
        layer_norm(r, g2b, be2b, o[:], eng=nc.vector)
        nc.sync.dma_start(out[t1t * P:(t1t + 1) * P, :], o[:])

    # ---- attention (t1-block outer so downstream work pipelines) ----
    for t1b in range(NB1):
        t1s = slice(t1b * 512, (t1b + 1) * 512)
        zall = norm.tile([8, 512], F32, tag="zall")
        cxu = {}
        for pair in range(NPAIR):
            hA, hB = 2 * pair, 2 * pair + 1
            cxA = ctx_ps.tile([DV1, 512], F32, tag="cxA")
            cxB = ctx_ps.tile([DV1, 512], F32, tag="cxB")
            for t2 in range(NT2):
                t2s = slice(t2 * P, (t2 + 1) * P)
                sAB = sc_ps.tile([P, 2, 512], F32, tag="s")
                nc.tensor.matmul(
                    sAB[:, 0, :], kt_sb[0:64, pair, t2s], qt_sb[0:64, pair, t1s],
                    start=True, stop=True, tile_position=(0, 0),
                )
                nc.tensor.matmul(
                    sAB[:, 1, :], kt_sb[64:128, pair, t2s], qt_sb[64:128, pair, t1s],
                    start=True, stop=True, tile_position=(64, 0),
                    skip_group_check=True,
                )
                eAB = exp_pool.tile([P, 2, 512], BF16, tag="e")
                nc.scalar.activation(eAB[:, :, :], sAB[:, :, :], AF.Exp)
                first, last = t2 == 0, t2 == NT2 - 1
                nc.tensor.matmul(
                    cxA[:, :], ve_sb[:, t2, hA, :], eAB[:, 0, :],
                    start=first, stop=last,
                )
                nc.tensor.matmul(
                    cxB[:, :], ve_sb[:, t2, hB, :], eAB[:, 1, :],
                    start=first, stop=last,
                )
            # evict unnormalized ctx (bf16) and gather Z rows (f32)
            for h, cx in ((hA, cxA), (hB, cxB)):
                cu = cxu_pool.tile([64, 512], BF16, tag="cu")
                nc.vector.tensor_copy(cu[:], cx[0:64, :])
                zst = norm.tile([P, 512], F32, tag="zst")
                nc.vector.tensor_copy(zst[64:65, :], cx[64:65, :])
                nc.sync.dma_start(zall[h:h + 1, :], zst[64:65, :])
                cxu[h] = cu
        # rz = 1/Z for all 8 heads at once (ACT, same table set as Exp)
        lz = norm.tile([8, 512], F32, tag="lz")
        nc.scalar.activation(lz[:], zall[:], AF.Ln)
        rzall = norm.tile([8, 512], BF16, tag="rzall")
        nc.scalar.activation(rzall[:], lz[:], AF.Exp, scale=-1.0)
        for h in range(H):
            pair, odd = divmod(h, 2)
            bch = mm_ps.tile([64, 512], F32, tag="mm")
            nc.tensor.matmul(
                bch[:], ind8_sb[:, h * DK:(h + 1) * DK], rzall[:, :],
                start=True, stop=True,
            )
            if not odd:
                nc.vector.tensor_tensor(
                    ctxT_sb[0:64, pair, t1s], cxu[h][:], bch[:], OP.mult
                )
            else:
                stg = work.tile([64, 512], BF16, tag="stg")
                nc.vector.tensor_tensor(stg[:], cxu[h][:], bch[:], OP.mult)
                nc.sync.dma_start(ctxT_sb[64:128, pair, t1s], stg[:])

    # post-attention + FFN emitted after BOTH attention blocks so the
    # attention pipeline keeps PE priority; this work gap-fills.
    for t1b in range(NB1):
        for t1t in range(t1b * 4, t1b * 4 + 4):
            post_attn(t1t)
        ffn1(t1b)
        for t1t in range(t1b * 4, t1b * 4 + 4):
            ffn2(t1t)


def _patch_act_tables():
    """Force every ACT op onto the natural_log_exp_and_others table set so
    the kernel pays one ACT_TABLE_LOAD instead of thrashing between the
    per-function default sets (Exp<->Ln cost 33 loads / 42us)."""
    import functools
    import concourse.hw_specs as hw_specs

    if getattr(hw_specs, "_nle_only", False):
        return
    orig = hw_specs.get_activation_tables

    @functools.cache
    def nle_only(arch):
        tabs = orig(arch)
        return {
            k: (v if k == "natural_log_exp_and_others" else set())
            for k, v in tabs.items()
        }

    hw_specs.get_activation_tables = nle_only
    hw_specs._nle_only = True
    # bacc imported the symbol directly
    if getattr(bacc, "get_activation_tables", None) is not None:
        bacc.get_activation_tables = nle_only


def build_program():
    _patch_act_tables()
    nc = bacc.Bacc("TRN2", target_bir_lowering=False, debug=False, num_devices=NCORES)
    io = {}
    io["xT"] = nc.dram_tensor("xT", [D, S], BF16, kind="ExternalInput").ap()
    io["xTo"] = nc.dram_tensor("xTo", [D, T1], BF16, kind="ExternalInput").ap()
    io["xo"] = nc.dram_tensor("xo", [T1, D], F32, kind="ExternalInput").ap()
    for name, shape in [
        ("wq", [D, D]), ("wk", [D, D]), ("wv", [D, D]), ("wo", [D, D]),
        ("w1", [D, DFF]), ("w2", [DFF, D]),
    ]:
        io[name] = nc.dram_tensor(name, shape, BF16, kind="ExternalInput").ap()
    for name, n in [
        ("bq", D), ("bk", D), ("bv", D), ("bo", D), ("b1", DFF), ("b2", D),
        ("g1", D), ("be1", D), ("g2", D), ("be2", D),
    ]:
        io[name] = nc.dram_tensor(name, [n], F32, kind="ExternalInput").ap()
    io["ind8"] = nc.dram_tensor("ind8", [8, D], BF16, kind="ExternalInput").ap()
    io["out"] = nc.dram_tensor("out", [T1, D], F32, kind="ExternalOutput").ap()

    with tile.TileContext(nc) as tc:
        with ExitStack() as ctx:
            emit(ctx, tc, io)
    nc.compile()
    return nc


def make_in_maps(x, Wq, bq, Wk, bk, Wv, bv, Wo, bo, W1, b1, W2, b2,
                 g1, be1, g2, be2):
    bf = ml_dtypes.bfloat16
    f32 = np.float32
    scale = 1.0 / np.sqrt(DK)
    shared = {
        "wq": (np.asarray(Wq, f32) * scale).astype(bf),
        "wk": np.asarray(Wk, f32).astype(bf),
        "wv": np.asarray(Wv, f32).astype(bf),
        "wo": np.asarray(Wo, f32).astype(bf),
        "w1": np.asarray(W1, f32).astype(bf),
        "w2": np.asarray(W2, f32).astype(bf),
        "bq": (np.asarray(bq, f32) * scale),
        "bk": np.asarray(bk, f32), "bv": np.asarray(bv, f32),
        "bo": np.asarray(bo, f32), "b1": np.asarray(b1, f32),
        "b2": np.asarray(b2, f32), "g1": np.asarray(g1, f32),
        "be1": np.asarray(be1, f32), "g2": np.asarray(g2, f32),
        "be2": np.asarray(be2, f32),
        "ind8": np.kron(np.eye(H, dtype=f32), np.ones((1, DK), f32)).astype(bf),
    }
    x = np.asarray(x, f32)
    in_maps = []
    for c in range(NCORES):
        b, half = divmod(c, 2)
        xb = x[b]                                  # [S, D] f32
        xTb = np.ascontiguousarray(xb.T).astype(bf)  # [D, S] bf16
        sl = slice(half * T1, (half + 1) * T1)
        m = dict(shared)
        m["xT"] = xTb
        m["xTo"] = np.ascontiguousarray(xTb[:, sl])
        m["xo"] = np.ascontiguousarray(xb[sl])
        in_maps.append(m)
    return in_maps


_prog_cache = {}


def get_program():
    if "nc" not in _prog_cache:
        _prog_cache["nc"] = build_program()
    return _prog_cache["nc"]


def kernel(**inputs) -> np.ndarray:
    nc = get_program()
    in_maps = make_in_maps(**inputs)
    res = run_bass_kernel_spmd(nc, in_maps, core_ids=list(range(NCORES)))
    out = np.empty((B, S, D), np.float32)
    for c in range(NCORES):
        b, half = divmod(c, 2)
        out[b, half * T1:(half + 1) * T1] = res.results[c]["out"]
    return out


if __name__ == "__main__":
    rng = np.random.default_rng(0)
    print("building program...")
    get_program()
    print("built")
